# revision 24
# baseline (speedup 1.0000x reference)
"""Causal multi-head attention (B=2, T=2048, DIM=1024, H=16) on 8 TRN2 cores.

Sharding: core c handles batch b = c // 4 and head-group g = c % 4 (4 heads,
head-dim slice of 256).  Each core computes QKV projections for its heads,
causal attention, and a partial output projection y_partial = o_g @ wo[:, g].T
of shape (2048, 1024).  Host sums the 4 partials per batch (the tensor-parallel
all-reduce, done on host as the unshard step).

All matmuls run in float32r (fp32 with 11-bit mantissa, full PE rate).  Inputs
are pre-rounded to fp32r on host; end-to-end error vs the fp32 reference is
~2e-4.

v2 pipeline: attention is ACT(exp)-bound (~900ns per kq-tile) while the
projections and output projection are PE-bound.  Instead of running the three
phases serially, projection quarters 1-3 and the per-group output projection
are emitted as a feeder queue whose units are pumped one-per-j-tile into the
attention loop, filling the PE gaps left while ACT computes exp.  Diagonal
score/attn-v matmuls are widened to >=256 moving elements (fp32r runs at 1/4
rate below 256); the widened region is zeroed by a shifted causal mask.

Device layout (T=2048 tokens of one batch, DH=256 head dims of one group):
  xt   [DIM, T]    x transposed (contraction over DIM needs DIM on partitions)
  qT/kT[128, 2, T] per pair p of 2 heads; partitions = 2x64 head dims
  v    [128, 16, 4, 65]  [t-tile, k-in-tile, head, head-dim + ones column]
  scores sT[k, q] via matmul(lhsT=kT, rhs=qT); softmax without max-subtraction
  (scores ~N(0,1)); denominator accumulated by the ones column of v during
  attn@v; normalization applied to oT via ones-outer-product broadcast.
"""

import sys

sys.path.insert(0, "/opt/trn_rl_repo")

import numpy as np

B, T, DIM, H = 2, 2048, 1024, 16
HD = DIM // H          # 64
NCORES = 8
GROUPS = 4             # head-groups (one per core pair-of-batches)
GH = H // GROUPS       # 4 heads per group
DH = GH * HD           # 256 head dims per group
NPAIR = 2              # pairs of heads per group (2 heads = 128 partitions)
TT = T // 128          # 16 t-tiles
TG = T // 512          # 4 q-groups of 512


def _round_f32r(x: np.ndarray) -> np.ndarray:
    """Round fp32 to fp32r (11-bit mantissa, RNE) as the PE expects."""
    u = np.ascontiguousarray(x, np.float32).view(np.uint32).astype(np.uint64)
    u = (u + 0x800 + ((u >> 12) & 1)) & 0xFFFFF000
    return u.astype(np.uint32).view(np.float32)


def _build_program(loop=1):
    import concourse.bass as bass
    import concourse.tile as tile
    from concourse import bacc, mybir
    from concourse.bass import _add_dep_helper

    F32 = mybir.dt.float32
    F32R = mybir.dt.float32r
    AF = mybir.ActivationFunctionType

    nc = bacc.Bacc("TRN2", target_bir_lowering=False, debug=False,
                   num_devices=NCORES)

    xt_d = nc.dram_tensor("xt", [DIM, T], F32R, kind="ExternalInput")
    wqt_d = nc.dram_tensor("wqt", [DIM, DH], F32R, kind="ExternalInput")
    wkt_d = nc.dram_tensor("wkt", [DIM, DH], F32R, kind="ExternalInput")
    wvt_d = nc.dram_tensor("wvt", [DIM, DH], F32R, kind="ExternalInput")
    wot_d = nc.dram_tensor("wot", [DH, DIM], F32R, kind="ExternalInput")
    y_d = nc.dram_tensor("y", [T, DIM], F32, kind="ExternalOutput")

    KO = DIM // 128  # 8 contraction chunks

    with tile.TileContext(nc) as tc:
        with (
            tc.tile_pool(name="singles", bufs=1) as singles,
            tc.tile_pool(name="workp", bufs=4) as workp,
            tc.tile_pool(name="worky", bufs=4) as worky,
            tc.tile_pool(name="tiny", bufs=3) as tiny,
            tc.tile_pool(name="psS", bufs=2, space="PSUM") as psS,
            tc.tile_pool(name="psO", bufs=2, space="PSUM") as psO,
            tc.tile_pool(name="psA", bufs=2, space="PSUM") as psA,
            tc.tile_pool(name="dramp", bufs=2, space="DRAM") as dramp,
        ):
            # ---- persistent SBUF tensors ----
            qT = singles.tile([128, NPAIR, T], F32R)
            kT = singles.tile([128, NPAIR, T], F32R)
            v = singles.tile([128, TT, GH, HD + 1], F32R)
            oT = singles.tile([128, NPAIR, T], F32R)

            # keep 1 where q - k >= 0 (k on partitions, q on free), else 0
            mask01 = singles.tile([128, 128], F32)
            nc.gpsimd.memset(mask01[:], 1.0)
            nc.gpsimd.affine_select(
                out=mask01[:], in_=mask01[:],
                compare_op=mybir.AluOpType.is_ge, fill=0.0,
                base=0, pattern=[[1, 128]], channel_multiplier=-1,
            )
            # widened diagonal mask: keep 1 where (q_off - 128) - k >= 0 over a
            # 256-wide region (first 128 columns fully masked)
            maskW = singles.tile([128, 256], F32)
            nc.gpsimd.memset(maskW[:], 1.0)
            nc.gpsimd.affine_select(
                out=maskW[:], in_=maskW[:],
                compare_op=mybir.AluOpType.is_ge, fill=0.0,
                base=-128, pattern=[[1, 256]], channel_multiplier=-1,
            )
            ones_f = singles.tile([128, HD], F32)
            nc.vector.memset(ones_f[:], 1.0)
            onesK = singles.tile([1, HD], F32R)
            nc.vector.tensor_copy(onesK[:], ones_f[0:1, :])
            # ones column of v (denominator accumulator)
            for h in range(GH):
                nc.vector.tensor_copy(v[:, :, h, HD:HD + 1], ones_f[:, 0:TT, None])
            # warm the ACT exp table during the initial DMA
            dummy = singles.tile([128, 1], F32)
            nc.scalar.activation(dummy[:], ones_f[:, 0:1], AF.Exp)

            # ---- device-side repetition for timing (loop > 1) ----
            for _it in range(loop):
              with (tc.tile_pool(name=f"wpool{_it}", bufs=1) as wpool,
                    tc.tile_pool(name=f"xqpool{_it}", bufs=3) as xqpool):
                xt_r = xt_d.rearrange("(ko p) t -> p ko t", p=128)
                wqt_sb = wpool.tile([128, KO, DH], F32R)
                wkt_sb = wpool.tile([128, KO, DH], F32R)
                wvt_sb = wpool.tile([128, KO, DH], F32R)
                wot_sb = wpool.tile([128, DH // 128, DIM], F32R)
                wqt_r = wqt_d.rearrange("(ko p) d -> p ko d", p=128)
                wkt_r = wkt_d.rearrange("(ko p) d -> p ko d", p=128)
                wvt_r = wvt_d.rearrange("(ko p) d -> p ko d", p=128)
                wot_r = wot_d.rearrange("(ko p) j -> p ko j", p=128)
                xq = [xqpool.tile([128, KO, 512], F32R, tag="xq",
                                  name=f"xq{_it}_{i}") for i in range(4)]
                # emission order = HWDGE descriptor-gen priority; the critical
                # first-quarter set runs unchained (parallel queues), bulk xq
                # quarters chain behind it so they don't steal bandwidth
                nc.sync.dma_start(wqt_sb[:, 0:4, 0:128], wqt_r[:, 0:4, 0:128])
                sub0 = []
                sub0.append(nc.sync.dma_start(xq[0][:, 0:2, :],
                                              xt_r[:, 0:2, 0:512]))
                sub0.append(nc.sync.dma_start(xq[0][:, 2:4, :],
                                              xt_r[:, 2:4, 0:512]))
                nc.sync.dma_start(wkt_sb[:, 0:4, 0:128], wkt_r[:, 0:4, 0:128])
                nc.sync.dma_start(wqt_sb[:, 4:8, 0:128], wqt_r[:, 4:8, 0:128])
                sub0.append(nc.sync.dma_start(xq[0][:, 4:6, :],
                                              xt_r[:, 4:6, 0:512]))
                sub0.append(nc.sync.dma_start(xq[0][:, 6:8, :],
                                              xt_r[:, 6:8, 0:512]))
                nc.sync.dma_start(wkt_sb[:, 4:8, 0:128], wkt_r[:, 4:8, 0:128])
                nc.sync.dma_start(wvt_sb, wvt_r)
                nc.sync.dma_start(wqt_sb[:, :, 128:DH], wqt_r[:, :, 128:DH])
                nc.sync.dma_start(wkt_sb[:, :, 128:DH], wkt_r[:, :, 128:DH])
                nc.sync.dma_start(wot_sb, wot_r)
                prev = sub0[-1]
                for quar in range(1, 4):
                    d = nc.sync.dma_start(
                        xq[quar], xt_r[:, :, 512 * quar:512 * (quar + 1)])
                    _add_dep_helper(d.ins, prev.ins, sync=True,
                                    reason="chain xt quarter loads")
                    prev = d

                # ---------- projection / output-projection work units ----------
                accs = {}

                def qk_half(quar, wsb, widx, dst, p, half, eng):
                    qsl = slice(512 * quar, 512 * (quar + 1))
                    key = (widx, quar, p)
                    if half == 0:
                        accs[key] = psA.tile(
                            [128, 512], F32, tag="acc",
                            name=f"qk{_it}_{widx}_{quar}_{p}")
                    acc = accs.pop(key) if half == 1 else accs[key]
                    for ko in range(4 * half, 4 * half + 4):
                        nc.tensor.matmul(
                            acc[:], wsb[:, ko, 128 * p:128 * (p + 1)],
                            xq[quar][:, ko, :],
                            start=(ko == 0), stop=(ko == KO - 1))
                    if half == 1:
                        if eng is nc.scalar:
                            nc.scalar.copy(dst[:, p, qsl], acc[:])
                        else:
                            eng.tensor_copy(dst[:, p, qsl], acc[:])

                def v_unit(tt, eng):
                    quar = tt // 4
                    acc = psA.tile([128, DH], F32, tag="acc",
                                   name=f"vac{_it}_{tt}")
                    for ko in range(KO):
                        nc.tensor.matmul(
                            acc[:],
                            xq[quar][:, ko, 128 * (tt % 4):128 * (tt % 4 + 1)],
                            wvt_sb[:, ko, :],
                            start=(ko == 0), stop=(ko == KO - 1))
                    if eng is nc.scalar:
                        nc.scalar.copy(v[:, tt, :, 0:HD],
                                       acc[:].rearrange("p (h d) -> p h d", h=GH))
                    else:
                        eng.tensor_copy(v[:, tt, :, 0:HD],
                                        acc[:].rearrange("p (h d) -> p h d", h=GH))

                def y_unit(tt, jh, tail=False):
                    acc = psA.tile([128, 512], F32, tag="acc",
                                   name=f"yac{_it}_{tt}_{jh}")
                    for p in range(NPAIR):
                        nc.tensor.matmul(
                            acc[:],
                            oT[:, p, 128 * tt:128 * (tt + 1)],
                            wot_sb[:, p, 512 * jh:512 * (jh + 1)],
                            start=(p == 0), stop=(p == NPAIR - 1))
                    ysb = worky.tile([128, 512], F32, tag="ysb", bufs=6,
                                     name=f"ysb{_it}_{tt}_{jh}")
                    # at the kernel tail ACT is done with exp: split drains
                    # across DVE and ACT so the last y tiles pipeline
                    if tail and jh == 1:
                        nc.scalar.copy(ysb[:], acc[:])
                    else:
                        nc.vector.tensor_copy(ysb[:], acc[:])
                    nc.sync.dma_start(
                        y_d[128 * tt:128 * (tt + 1),
                            512 * jh:512 * (jh + 1)], ysb[:])

                # mandatory feeder: (key, fn) sorted FIFO; key = (G, p) means
                # "must be emitted before attention pair (G, p) starts"
                mand = []
                opty = []
                for p in range(1, NPAIR):
                    for widx, wsb, dst in ((0, wqt_sb, qT), (1, wkt_sb, kT)):
                        for half in range(2):
                            mand.append(((0, p), lambda w=wsb, i=widx, d=dst,
                                         pp=p, h=half: qk_half(0, w, i, d, pp,
                                                               h, nc.vector)))
                for quar in range(1, 4):
                    for widx, wsb, dst in ((0, wqt_sb, qT), (1, wkt_sb, kT)):
                        for half in range(2):
                            mand.append(((quar, 0), lambda q=quar, w=wsb,
                                         i=widx, d=dst, h=half: qk_half(
                                             q, w, i, d, 0, h, nc.vector)))
                    for tt in range(4 * quar, 4 * quar + 4):
                        mand.append(((quar, 0), lambda t=tt: v_unit(
                            t, nc.vector)))
                    for widx, wsb, dst in ((0, wqt_sb, qT), (1, wkt_sb, kT)):
                        for half in range(2):
                            mand.append(((quar, 1), lambda q=quar, w=wsb,
                                         i=widx, d=dst, h=half: qk_half(
                                             q, w, i, d, 1, h, nc.vector)))

                def flush_mand(upto):
                    while mand and mand[0][0] <= upto:
                        mand.pop(0)[1]()

                # ---------- prolog: quarter-0 q/k projections for pair 0 ----
                # (v quarter-0 is emitted inline in the first attention pair's
                # j-loop so scores/exp can start before wv even arrives)
                for widx, wsb, dst in ((0, wqt_sb, qT), (1, wkt_sb, kT)):
                    for half in range(2):
                        qk_half(0, wsb, widx, dst, 0, half, nc.scalar)

                # ---------- attention, feeder-interleaved ----------
                pair_seq = [(G, p) for G in range(TG) for p in range(NPAIR)]
                steps_left = {"n": sum(4 * G + 5 for G, p in pair_seq)}
                for pi, (G, p) in enumerate(pair_seq):
                    if True:
                        flush_mand((G, p))
                        # spread units due before the NEXT pair evenly across
                        # this pair's j-steps (avoids PE stalling on DMAs for
                        # far-future quarters and block-flushes at boundaries)
                        nxt = pair_seq[pi + 1] if pi + 1 < len(pair_seq) else (99, 99)
                        n_due = sum(1 for k, _ in mand if k <= nxt)
                        njt = 4 * G + 4
                        state = {"mand": 0}

                        def pump(step, n_due=n_due, njt=njt, nxt=nxt,
                                 state=state):
                            steps_left["n"] -= 1
                            target = ((step + 1) * n_due + njt - 1) // njt
                            emitted = False
                            while (state["mand"] < target and mand
                                   and mand[0][0] <= nxt):
                                mand.pop(0)[1]()
                                state["mand"] += 1
                                emitted = True
                            # release y units only when the backlog is large
                            # relative to remaining steps, reserving enough to
                            # fill the ACT-bound final pairs' PE gaps
                            if (not emitted and opty
                                    and len(opty) * 3 >= steps_left["n"]):
                                opty.pop(0)()

                        hA, hB = 2 * p, 2 * p + 1
                        oA = psO.tile([HD + 1, 512], F32, tag="o",
                                      name=f"oA_{_it}_{p}_{G}")
                        oB = psO.tile([HD + 1, 512], F32, tag="o",
                                      name=f"oB_{_it}_{p}_{G}")
                        njt = 4 * G + 4  # causal: k-tiles 0 .. 4G+3
                        for j in range(njt):
                            dlt = j - 4 * G
                            # widen the last diagonal tile to 256 moving
                            # elements: fp32r matmuls below 256 run at 1/4 rate
                            off = min(max(0, dlt), 2) * 128
                            qs = slice(512 * G + off, 512 * (G + 1))
                            ks = slice(128 * j, 128 * (j + 1))
                            sAB = psS.tile([128, 1024], F32, tag="sc",
                                           name=f"s{_it}_{p}_{G}_{j}")
                            nc.tensor.matmul(sAB[:, off:512],
                                             kT[0:64, p, ks], qT[0:64, p, qs],
                                             start=True, stop=True)
                            nc.tensor.matmul(sAB[:, 512 + off:1024],
                                             kT[64:128, p, ks],
                                             qT[64:128, p, qs],
                                             start=True, stop=True)
                            pAB = workp.tile([128, 1024], F32R, tag="pT",
                                             name=f"p{_it}_{p}_{G}_{j}")
                            sr = sAB[:].rearrange("par (two q) -> par two q",
                                                  two=2)[:, :, off:]
                            pr = pAB[:].rearrange("par (two q) -> par two q",
                                                  two=2)[:, :, off:]
                            nc.scalar.activation(pr, sr, AF.Exp)
                            if G == 0 and p == 0:
                                v_unit(j, nc.vector)
                            pump(j)
                            if dlt >= 0:  # diagonal: multiplicative mask
                                w = 256 if dlt == 3 else 128
                                msk = maskW if dlt == 3 else mask01
                                dst = pAB[:].rearrange(
                                    "par (two q) -> par two q",
                                    two=2)[:, :, off:off + w]
                                nc.vector.tensor_mul(
                                    dst, dst,
                                    msk[:, None, 0:w].to_broadcast(
                                        (128, 2, w)))
                            nc.tensor.matmul(oA[:, off:],
                                             v[:, j, hA, :], pAB[:, off:512],
                                             start=(j == 0),
                                             stop=(j == njt - 1))
                            nc.tensor.matmul(oB[:, off:],
                                             v[:, j, hB, :],
                                             pAB[:, 512 + off:1024],
                                             start=(j == 0),
                                             stop=(j == njt - 1))
                        # drain o psum to SBUF immediately; normalization is
                        # off the critical path and avoids PE and PSUM
                        for sigma, po in ((0, oA), (1, oB)):
                            if G == TG - 1:
                                # tail-critical: reciprocal straight from
                                # psum (DVE) in parallel with the o drain on
                                # the now-idle ACT, then broadcast 1/denom to
                                # 64 rows as a K=1 PE outer product (no DMA
                                # latency); DVE mul reads SBUF x PSUM
                                r0 = tiny.tile([1, 512], F32R, tag="r0")
                                with nc.allow_low_precision(
                                        reason="f32r 1/denom, PE broadcast"):
                                    nc.vector.reciprocal(r0[:],
                                                         po[HD:HD + 1, :])
                                oU = tiny.tile([HD, 512], F32, tag="oU")
                                nc.scalar.copy(oU[:], po[0:HD, :])
                                Rps = psA.tile([HD, 512], F32, tag="acc",
                                               name=f"rb{_it}_{p}_{sigma}")
                                nc.tensor.matmul(Rps[:], onesK[:], r0[:],
                                                 start=True, stop=True)
                                nc.vector.tensor_mul(
                                    oT[64 * sigma:64 * (sigma + 1), p,
                                       512 * G:512 * (G + 1)],
                                    oU[:], Rps[:])
                                continue
                            oU = tiny.tile([HD + 1, 512], F32, tag="oU")
                            nc.vector.tensor_copy(oU[:], po[:])
                            r0 = tiny.tile([1, 512], F32R, tag="r0")
                            with nc.allow_low_precision(
                                    reason="f32r 1/denom feeds DMA broadcast"):
                                nc.vector.reciprocal(r0[:], oU[HD:HD + 1, :])
                            # broadcast 1/denom to 64 rows via DRAM bounce
                            # (off the critical path for non-final groups)
                            rdr = dramp.tile([1, 512], F32R)
                            nc.sync.dma_start(rdr[:], r0[:])
                            Rsb = tiny.tile([HD, 512], F32R, tag="Rsb")
                            rdrap = rdr[:]
                            bcast = bass.AP(tensor=rdrap.tensor,
                                            offset=rdrap.offset,
                                            ap=[[0, HD]] + list(rdrap.ap)[1:])
                            nc.sync.dma_start(Rsb[:], bcast)
                            # normalize on the idle Pool engine
                            nc.gpsimd.tensor_mul(
                                oT[64 * sigma:64 * (sigma + 1), p,
                                   512 * G:512 * (G + 1)],
                                oU[0:HD, :], Rsb[:])
                        pump(njt)
                    # output projection for this q-group feeds later PE gaps
                    if p == NPAIR - 1:
                        tail = G == TG - 1
                        for tt in range(4 * G, 4 * (G + 1)):
                            for jh in range(2):
                                opty.append(lambda t=tt, j=jh, tl=tail:
                                            y_unit(t, j, tl))

                # ---------- epilogue: drain remaining feeder work ----------
                flush_mand((99, 99))
                while opty:
                    opty.pop(0)()

    nc.compile()
    return nc


_RUNNER = None


def _make_pjrt_runner(nc):
    """Wrap a compiled Bass program as an 8-core PJRT callable."""
    import jax
    import numpy as _np
    from jax.sharding import Mesh, PartitionSpec
    from jax.experimental.shard_map import shard_map
    from concourse import bass2jax, mybir
    from concourse.bass2jax import (_bass_exec_p, install_neuronx_cc_hook,
                                    partition_id_tensor)

    install_neuronx_cc_hook()

    partition_name = (nc.partition_id_tensor.name
                      if nc.partition_id_tensor else None)
    in_names, out_names, out_avals = [], [], []
    for alloc in nc.m.functions[0].allocations:
        if not isinstance(alloc, mybir.MemoryLocationSet):
            continue
        if not alloc.memorylocations:
            continue
        name = alloc.memorylocations[0].name
        if alloc.kind == "ExternalInput":
            if name != partition_name:
                in_names.append(name)
        elif alloc.kind == "ExternalOutput":
            out_names.append(name)
            out_avals.append(jax.core.ShapedArray(
                tuple(alloc.tensor_shape), mybir.dt.np(alloc.dtype)))
    n_params = len(in_names)
    n_outs = len(out_names)
    zero_shapes = [(a.shape, a.dtype) for a in out_avals]
    all_in_names = in_names + out_names
    if partition_name is not None:
        all_in_names = all_in_names + [partition_name]

    def _body(*args):
        operands = list(args)
        if partition_name is not None:
            operands.append(partition_id_tensor())
        outs = _bass_exec_p.bind(
            *operands,
            out_avals=tuple(out_avals),
            in_names=tuple(all_in_names),
            out_names=tuple(out_names),
            lowering_input_output_aliases=(),
            sim_require_finite=True,
            sim_require_nnan=True,
            nc=nc,
        )
        return tuple(outs)

    devices = jax.devices()[:NCORES]
    mesh = Mesh(np.asarray(devices), ("core",))
    sharded = jax.jit(
        shard_map(_body, mesh=mesh,
                  in_specs=(PartitionSpec("core"),) * (n_params + n_outs),
                  out_specs=(PartitionSpec("core"),) * n_outs,
                  check_rep=False),
        keep_unused=True,
    )

    def run(in_maps):
        concat_in = [
            _np.concatenate([_np.asarray(in_maps[c][n]) for c in range(NCORES)],
                            axis=0)
            for n in in_names
        ]
        concat_zeros = [
            _np.zeros((NCORES * s[0], *s[1:]), d) for (s, d) in zero_shapes
        ]
        out_arrs = sharded(*concat_in, *concat_zeros)
        return [
            {
                n: _np.asarray(out_arrs[i]).reshape(NCORES, *out_avals[i].shape)[c]
                for i, n in enumerate(out_names)
            }
            for c in range(NCORES)
        ]

    internals = dict(nc=nc, body=_body, mesh=mesh, in_names=in_names,
                     out_names=out_names, zero_shapes=zero_shapes,
                     n_params=n_params)
    return run, in_names, internals


def _get_runner():
    """Build the Bass program once and return a cached 8-core PJRT callable."""
    global _RUNNER, _INTERNALS
    if _RUNNER is not None:
        return _RUNNER
    run, in_names, internals = _make_pjrt_runner(_build_program())
    _INTERNALS = internals
    _RUNNER = (run, in_names)
    return _RUNNER


def _make_in_maps(x, wq, wk, wv, wo):
    x = np.asarray(x, np.float32)
    wq_s = np.asarray(wq, np.float32) * (1.0 / np.sqrt(HD))  # fold score scale
    wk = np.asarray(wk, np.float32)
    wv = np.asarray(wv, np.float32)
    wo = np.asarray(wo, np.float32)

    xt_b = [_round_f32r(x[b].T) for b in range(B)]
    in_maps = []
    for c in range(NCORES):
        b, g = c // GROUPS, c % GROUPS
        sl = slice(DH * g, DH * (g + 1))
        in_maps.append({
            "xt": xt_b[b],
            "wqt": _round_f32r(wq_s[sl, :].T),
            "wkt": _round_f32r(wk[sl, :].T),
            "wvt": _round_f32r(wv[sl, :].T),
            "wot": _round_f32r(wo[:, sl].T),
        })
    return in_maps


def kernel(x, wq, wk, wv, wo):
    run, _ = _get_runner()
    results = run(_make_in_maps(x, wq, wk, wv, wo))
    y = np.zeros((B, T, DIM), np.float32)
    for c in range(NCORES):
        y[c // GROUPS] += results[c]["y"]
    return y


# revision 26
# speedup vs baseline: 1.2175x; 1.2175x over previous
"""Causal multi-head attention (B=2, T=2048, DIM=1024, H=16) on 8 TRN2 cores.

Sharding: core c handles batch b = c // 4 and head-group g = c % 4 (4 heads,
head-dim slice of 256).  Each core computes QKV projections for its heads,
causal attention, and a partial output projection y_partial = o_g @ wo[:, g].T
of shape (2048, 1024).  Host sums the 4 partials per batch (the tensor-parallel
all-reduce, done on host as the unshard step).

All matmuls run in float32r (fp32 with 11-bit mantissa, full PE rate).  Inputs
are pre-rounded to fp32r on host; end-to-end error vs the fp32 reference is
~2e-4.

v2 pipeline: attention is ACT(exp)-bound (~900ns per kq-tile) while the
projections and output projection are PE-bound.  Instead of running the three
phases serially, projection quarters 1-3 and the per-group output projection
are emitted as a feeder queue whose units are pumped one-per-j-tile into the
attention loop, filling the PE gaps left while ACT computes exp.  Diagonal
score/attn-v matmuls are widened to >=256 moving elements (fp32r runs at 1/4
rate below 256); the widened region is zeroed by a shifted causal mask.

Device layout (T=2048 tokens of one batch, DH=256 head dims of one group):
  xt   [DIM, T]    x transposed (contraction over DIM needs DIM on partitions)
  qT/kT[128, 2, T] per pair p of 2 heads; partitions = 2x64 head dims
  v    [128, 16, 4, 65]  [t-tile, k-in-tile, head, head-dim + ones column]
  scores sT[k, q] via matmul(lhsT=kT, rhs=qT); softmax without max-subtraction
  (scores ~N(0,1)); denominator accumulated by the ones column of v during
  attn@v; normalization applied to oT via ones-outer-product broadcast.
"""

import sys

sys.path.insert(0, "/opt/trn_rl_repo")

import numpy as np

B, T, DIM, H = 2, 2048, 1024, 16
HD = DIM // H          # 64
NCORES = 8
GROUPS = 4             # head-groups (one per core pair-of-batches)
GH = H // GROUPS       # 4 heads per group
DH = GH * HD           # 256 head dims per group
NPAIR = 2              # pairs of heads per group (2 heads = 128 partitions)
TT = T // 128          # 16 t-tiles
TG = T // 512          # 4 q-groups of 512


def _round_f32r(x: np.ndarray) -> np.ndarray:
    """Round fp32 to fp32r (11-bit mantissa, RNE) as the PE expects."""
    u = np.ascontiguousarray(x, np.float32).view(np.uint32).astype(np.uint64)
    u = (u + 0x800 + ((u >> 12) & 1)) & 0xFFFFF000
    return u.astype(np.uint32).view(np.float32)


def _build_program(loop=1):
    import concourse.bass as bass
    import concourse.tile as tile
    from concourse import bacc, mybir
    from concourse.bass import _add_dep_helper

    F32 = mybir.dt.float32
    F32R = mybir.dt.float32r
    AF = mybir.ActivationFunctionType

    nc = bacc.Bacc("TRN2", target_bir_lowering=False, debug=False,
                   num_devices=NCORES)

    xt_d = nc.dram_tensor("xt", [DIM, T], F32R, kind="ExternalInput")
    wqt_d = nc.dram_tensor("wqt", [DIM, DH], F32R, kind="ExternalInput")
    wkt_d = nc.dram_tensor("wkt", [DIM, DH], F32R, kind="ExternalInput")
    wvt_d = nc.dram_tensor("wvt", [DIM, DH], F32R, kind="ExternalInput")
    wot_d = nc.dram_tensor("wot", [DH, DIM], F32R, kind="ExternalInput")
    y_d = nc.dram_tensor("y", [T, DIM], F32, kind="ExternalOutput")

    KO = DIM // 128  # 8 contraction chunks

    with tile.TileContext(nc) as tc:
        with (
            tc.tile_pool(name="singles", bufs=1) as singles,
            tc.tile_pool(name="workp", bufs=4) as workp,
            tc.tile_pool(name="worky", bufs=4) as worky,
            tc.tile_pool(name="tiny", bufs=3) as tiny,
            tc.tile_pool(name="psS", bufs=2, space="PSUM") as psS,
            tc.tile_pool(name="psO", bufs=2, space="PSUM") as psO,
            tc.tile_pool(name="psA", bufs=2, space="PSUM") as psA,
            tc.tile_pool(name="dramp", bufs=2, space="DRAM") as dramp,
        ):
            # ---- persistent SBUF tensors ----
            qT = singles.tile([128, NPAIR, T], F32R)
            kT = singles.tile([128, NPAIR, T], F32R)
            v = singles.tile([128, TT, GH, HD + 1], F32R)
            oT = singles.tile([128, NPAIR, T], F32R)

            # keep 1 where q - k >= 0 (k on partitions, q on free), else 0
            mask01 = singles.tile([128, 128], F32)
            nc.gpsimd.memset(mask01[:], 1.0)
            nc.gpsimd.affine_select(
                out=mask01[:], in_=mask01[:],
                compare_op=mybir.AluOpType.is_ge, fill=0.0,
                base=0, pattern=[[1, 128]], channel_multiplier=-1,
            )
            # widened diagonal mask: keep 1 where (q_off - 128) - k >= 0 over a
            # 256-wide region (first 128 columns fully masked)
            maskW = singles.tile([128, 256], F32)
            nc.gpsimd.memset(maskW[:], 1.0)
            nc.gpsimd.affine_select(
                out=maskW[:], in_=maskW[:],
                compare_op=mybir.AluOpType.is_ge, fill=0.0,
                base=-128, pattern=[[1, 256]], channel_multiplier=-1,
            )
            ones_f = singles.tile([128, HD], F32)
            nc.vector.memset(ones_f[:], 1.0)
            onesK = singles.tile([1, HD], F32R)
            nc.vector.tensor_copy(onesK[:], ones_f[0:1, :])
            # ones column of v (denominator accumulator)
            for h in range(GH):
                nc.vector.tensor_copy(v[:, :, h, HD:HD + 1], ones_f[:, 0:TT, None])
            # warm the ACT exp table during the initial DMA
            dummy = singles.tile([128, 1], F32)
            nc.scalar.activation(dummy[:], ones_f[:, 0:1], AF.Exp)

            # ---- device-side repetition for timing (loop > 1) ----
            for _it in range(loop):
              with (tc.tile_pool(name=f"wpool{_it}", bufs=1) as wpool,
                    tc.tile_pool(name=f"xqpool{_it}", bufs=3) as xqpool):
                xt_r = xt_d.rearrange("(ko p) t -> p ko t", p=128)
                wqt_sb = wpool.tile([128, KO, DH], F32R)
                wkt_sb = wpool.tile([128, KO, DH], F32R)
                wvt_sb = wpool.tile([128, KO, DH], F32R)
                wot_sb = wpool.tile([128, DH // 128, DIM], F32R)
                wqt_r = wqt_d.rearrange("(ko p) d -> p ko d", p=128)
                wkt_r = wkt_d.rearrange("(ko p) d -> p ko d", p=128)
                wvt_r = wvt_d.rearrange("(ko p) d -> p ko d", p=128)
                wot_r = wot_d.rearrange("(ko p) j -> p ko j", p=128)
                xq = [xqpool.tile([128, KO, 512], F32R, tag="xq",
                                  name=f"xq{_it}_{i}") for i in range(4)]
                # emission order = HWDGE descriptor-gen priority; the critical
                # first-quarter set runs unchained (parallel queues), bulk xq
                # quarters chain behind it so they don't steal bandwidth
                nc.sync.dma_start(wqt_sb[:, 0:4, 0:128], wqt_r[:, 0:4, 0:128])
                sub0 = []
                sub0.append(nc.sync.dma_start(xq[0][:, 0:2, :],
                                              xt_r[:, 0:2, 0:512]))
                sub0.append(nc.sync.dma_start(xq[0][:, 2:4, :],
                                              xt_r[:, 2:4, 0:512]))
                nc.sync.dma_start(wkt_sb[:, 0:4, 0:128], wkt_r[:, 0:4, 0:128])
                nc.sync.dma_start(wqt_sb[:, 4:8, 0:128], wqt_r[:, 4:8, 0:128])
                sub0.append(nc.sync.dma_start(xq[0][:, 4:6, :],
                                              xt_r[:, 4:6, 0:512]))
                sub0.append(nc.sync.dma_start(xq[0][:, 6:8, :],
                                              xt_r[:, 6:8, 0:512]))
                nc.sync.dma_start(wkt_sb[:, 4:8, 0:128], wkt_r[:, 4:8, 0:128])
                nc.sync.dma_start(wvt_sb, wvt_r)
                nc.sync.dma_start(wqt_sb[:, :, 128:DH], wqt_r[:, :, 128:DH])
                nc.sync.dma_start(wkt_sb[:, :, 128:DH], wkt_r[:, :, 128:DH])
                nc.sync.dma_start(wot_sb, wot_r)
                prev = sub0[-1]
                for quar in range(1, 4):
                    for kh in range(2):
                        d = nc.sync.dma_start(
                            xq[quar][:, 4 * kh:4 * kh + 4, :],
                            xt_r[:, 4 * kh:4 * kh + 4,
                                 512 * quar:512 * (quar + 1)])
                        _add_dep_helper(d.ins, prev.ins, sync=True,
                                        reason="chain xt quarter loads")
                        prev = d

                # ---------- projection / output-projection work units ----------
                accs = {}

                def qk_half(quar, wsb, widx, dst, p, half, eng):
                    qsl = slice(512 * quar, 512 * (quar + 1))
                    key = (widx, quar, p)
                    if half == 0:
                        accs[key] = psA.tile(
                            [128, 512], F32, tag="acc",
                            name=f"qk{_it}_{widx}_{quar}_{p}")
                    acc = accs.pop(key) if half == 1 else accs[key]
                    for ko in range(4 * half, 4 * half + 4):
                        nc.tensor.matmul(
                            acc[:], wsb[:, ko, 128 * p:128 * (p + 1)],
                            xq[quar][:, ko, :],
                            start=(ko == 0), stop=(ko == KO - 1))
                    if half == 1:
                        if eng is nc.scalar:
                            nc.scalar.copy(dst[:, p, qsl], acc[:])
                        else:
                            eng.tensor_copy(dst[:, p, qsl], acc[:])

                def v_unit(tt, eng):
                    quar = tt // 4
                    acc = psA.tile([128, DH], F32, tag="acc",
                                   name=f"vac{_it}_{tt}")
                    for ko in range(KO):
                        nc.tensor.matmul(
                            acc[:],
                            xq[quar][:, ko, 128 * (tt % 4):128 * (tt % 4 + 1)],
                            wvt_sb[:, ko, :],
                            start=(ko == 0), stop=(ko == KO - 1))
                    if eng is nc.scalar:
                        nc.scalar.copy(v[:, tt, :, 0:HD],
                                       acc[:].rearrange("p (h d) -> p h d", h=GH))
                    else:
                        eng.tensor_copy(v[:, tt, :, 0:HD],
                                        acc[:].rearrange("p (h d) -> p h d", h=GH))

                def y_unit(tt, jh, tail=False):
                    acc = psA.tile([128, 512], F32, tag="acc",
                                   name=f"yac{_it}_{tt}_{jh}")
                    for p in range(NPAIR):
                        nc.tensor.matmul(
                            acc[:],
                            oT[:, p, 128 * tt:128 * (tt + 1)],
                            wot_sb[:, p, 512 * jh:512 * (jh + 1)],
                            start=(p == 0), stop=(p == NPAIR - 1))
                    ysb = worky.tile([128, 512], F32, tag="ysb", bufs=6,
                                     name=f"ysb{_it}_{tt}_{jh}")
                    # at the kernel tail ACT is done with exp: split drains
                    # across DVE and ACT so the last y tiles pipeline
                    if tail and jh == 1:
                        nc.scalar.copy(ysb[:], acc[:])
                    else:
                        nc.vector.tensor_copy(ysb[:], acc[:])
                    nc.sync.dma_start(
                        y_d[128 * tt:128 * (tt + 1),
                            512 * jh:512 * (jh + 1)], ysb[:])

                # mandatory feeder: (key, fn) sorted FIFO; key = (G, p) means
                # "must be emitted before attention pair (G, p) starts"
                mand = []
                opty = []
                for p in range(1, NPAIR):
                    for widx, wsb, dst in ((0, wqt_sb, qT), (1, wkt_sb, kT)):
                        for half in range(2):
                            mand.append(((0, p), lambda w=wsb, i=widx, d=dst,
                                         pp=p, h=half: qk_half(0, w, i, d, pp,
                                                               h, nc.vector)))
                for quar in range(1, 4):
                    for widx, wsb, dst in ((0, wqt_sb, qT), (1, wkt_sb, kT)):
                        for half in range(2):
                            mand.append(((quar, 0), lambda q=quar, w=wsb,
                                         i=widx, d=dst, h=half: qk_half(
                                             q, w, i, d, 0, h, nc.vector)))
                    for tt in range(4 * quar, 4 * quar + 4):
                        mand.append(((quar, 0), lambda t=tt: v_unit(
                            t, nc.vector)))
                    for widx, wsb, dst in ((0, wqt_sb, qT), (1, wkt_sb, kT)):
                        for half in range(2):
                            mand.append(((quar, 1), lambda q=quar, w=wsb,
                                         i=widx, d=dst, h=half: qk_half(
                                             q, w, i, d, 1, h, nc.vector)))

                def flush_mand(upto):
                    while mand and mand[0][0] <= upto:
                        mand.pop(0)[1]()

                # ---------- prolog: quarter-0 q/k projections for pair 0 ----
                # (v quarter-0 is emitted inline in the first attention pair's
                # j-loop so scores/exp can start before wv even arrives)
                for widx, wsb, dst in ((0, wqt_sb, qT), (1, wkt_sb, kT)):
                    for half in range(2):
                        qk_half(0, wsb, widx, dst, 0, half, nc.scalar)

                # ---------- attention, feeder-interleaved ----------
                pair_seq = [(G, p) for G in range(TG) for p in range(NPAIR)]
                steps_left = {"n": sum(4 * G + 5 for G, p in pair_seq)}
                for pi, (G, p) in enumerate(pair_seq):
                    if True:
                        flush_mand((G, p))
                        # spread units due before the NEXT pair evenly across
                        # this pair's j-steps (avoids PE stalling on DMAs for
                        # far-future quarters and block-flushes at boundaries)
                        nxt = pair_seq[pi + 1] if pi + 1 < len(pair_seq) else (99, 99)
                        n_due = sum(1 for k, _ in mand if k <= nxt)
                        njt = 4 * G + 4
                        state = {"mand": 0}

                        def pump(step, n_due=n_due, njt=njt, nxt=nxt,
                                 state=state):
                            steps_left["n"] -= 1
                            target = ((step + 1) * n_due + njt - 1) // njt
                            emitted = False
                            while (state["mand"] < target and mand
                                   and mand[0][0] <= nxt):
                                mand.pop(0)[1]()
                                state["mand"] += 1
                                emitted = True
                            # release y units only when the backlog is large
                            # relative to remaining steps, reserving enough to
                            # fill the ACT-bound final pairs' PE gaps
                            if (not emitted and opty
                                    and len(opty) * 3 >= steps_left["n"]):
                                opty.pop(0)()

                        hA, hB = 2 * p, 2 * p + 1
                        oA = psO.tile([HD + 1, 512], F32, tag="o",
                                      name=f"oA_{_it}_{p}_{G}")
                        oB = psO.tile([HD + 1, 512], F32, tag="o",
                                      name=f"oB_{_it}_{p}_{G}")
                        njt = 4 * G + 4  # causal: k-tiles 0 .. 4G+3
                        for j in range(njt):
                            dlt = j - 4 * G
                            # widen the last diagonal tile to 256 moving
                            # elements: fp32r matmuls below 256 run at 1/4 rate
                            off = min(max(0, dlt), 2) * 128
                            qs = slice(512 * G + off, 512 * (G + 1))
                            ks = slice(128 * j, 128 * (j + 1))
                            sAB = psS.tile([128, 1024], F32, tag="sc",
                                           name=f"s{_it}_{p}_{G}_{j}")
                            nc.tensor.matmul(sAB[:, off:512],
                                             kT[0:64, p, ks], qT[0:64, p, qs],
                                             start=True, stop=True)
                            nc.tensor.matmul(sAB[:, 512 + off:1024],
                                             kT[64:128, p, ks],
                                             qT[64:128, p, qs],
                                             start=True, stop=True)
                            pAB = workp.tile([128, 1024], F32R, tag="pT",
                                             name=f"p{_it}_{p}_{G}_{j}")
                            # contiguous exps (no strided AP): one for the
                            # full tile off-diagonal, two per-head on diagonal
                            if off == 0:
                                nc.scalar.activation(pAB[:], sAB[:], AF.Exp)
                            else:
                                nc.scalar.activation(pAB[:, off:512],
                                                     sAB[:, off:512], AF.Exp)
                                nc.scalar.activation(pAB[:, 512 + off:1024],
                                                     sAB[:, 512 + off:1024],
                                                     AF.Exp)
                            if G == 0 and p == 0:
                                v_unit(j, nc.vector)
                            pump(j)
                            if dlt >= 0:  # diagonal: multiplicative mask
                                w = 256 if dlt == 3 else 128
                                msk = maskW if dlt == 3 else mask01
                                dst = pAB[:].rearrange(
                                    "par (two q) -> par two q",
                                    two=2)[:, :, off:off + w]
                                nc.vector.tensor_mul(
                                    dst, dst,
                                    msk[:, None, 0:w].to_broadcast(
                                        (128, 2, w)))
                            nc.tensor.matmul(oA[:, off:],
                                             v[:, j, hA, :], pAB[:, off:512],
                                             start=(j == 0),
                                             stop=(j == njt - 1))
                            nc.tensor.matmul(oB[:, off:],
                                             v[:, j, hB, :],
                                             pAB[:, 512 + off:1024],
                                             start=(j == 0),
                                             stop=(j == njt - 1))
                        # drain o psum to SBUF immediately; normalization is
                        # off the critical path and avoids PE and PSUM
                        for sigma, po in ((0, oA), (1, oB)):
                            if G == TG - 1:
                                # tail-critical: reciprocal straight from
                                # psum (DVE) in parallel with the o drain on
                                # the now-idle ACT, then broadcast 1/denom to
                                # 64 rows as a K=1 PE outer product (no DMA
                                # latency); DVE mul reads SBUF x PSUM
                                r0 = tiny.tile([1, 512], F32R, tag="r0")
                                with nc.allow_low_precision(
                                        reason="f32r 1/denom, PE broadcast"):
                                    nc.vector.reciprocal(r0[:],
                                                         po[HD:HD + 1, :])
                                oU = tiny.tile([HD, 512], F32, tag="oU")
                                nc.scalar.copy(oU[:], po[0:HD, :])
                                Rps = psA.tile([HD, 512], F32, tag="acc",
                                               name=f"rb{_it}_{p}_{sigma}")
                                nc.tensor.matmul(Rps[:], onesK[:], r0[:],
                                                 start=True, stop=True)
                                nc.vector.tensor_mul(
                                    oT[64 * sigma:64 * (sigma + 1), p,
                                       512 * G:512 * (G + 1)],
                                    oU[:], Rps[:])
                                continue
                            oU = tiny.tile([HD + 1, 512], F32, tag="oU")
                            nc.vector.tensor_copy(oU[:], po[:])
                            r0 = tiny.tile([1, 512], F32R, tag="r0")
                            with nc.allow_low_precision(
                                    reason="f32r 1/denom feeds DMA broadcast"):
                                nc.vector.reciprocal(r0[:], oU[HD:HD + 1, :])
                            # broadcast 1/denom to 64 rows via DRAM bounce
                            # (off the critical path for non-final groups)
                            rdr = dramp.tile([1, 512], F32R)
                            nc.sync.dma_start(rdr[:], r0[:])
                            Rsb = tiny.tile([HD, 512], F32R, tag="Rsb")
                            rdrap = rdr[:]
                            bcast = bass.AP(tensor=rdrap.tensor,
                                            offset=rdrap.offset,
                                            ap=[[0, HD]] + list(rdrap.ap)[1:])
                            nc.sync.dma_start(Rsb[:], bcast)
                            # normalize on the idle Pool engine
                            nc.gpsimd.tensor_mul(
                                oT[64 * sigma:64 * (sigma + 1), p,
                                   512 * G:512 * (G + 1)],
                                oU[0:HD, :], Rsb[:])
                        pump(njt)
                    # output projection for this q-group feeds later PE gaps
                    if p == NPAIR - 1:
                        tail = G == TG - 1
                        for tt in range(4 * G, 4 * (G + 1)):
                            for jh in range(2):
                                opty.append(lambda t=tt, j=jh, tl=tail:
                                            y_unit(t, j, tl))

                # ---------- epilogue: drain remaining feeder work ----------
                flush_mand((99, 99))
                while opty:
                    opty.pop(0)()

    nc.compile()
    return nc


_RUNNER = None


def _make_pjrt_runner(nc):
    """Wrap a compiled Bass program as an 8-core PJRT callable."""
    import jax
    import numpy as _np
    from jax.sharding import Mesh, PartitionSpec
    from jax.experimental.shard_map import shard_map
    from concourse import bass2jax, mybir
    from concourse.bass2jax import (_bass_exec_p, install_neuronx_cc_hook,
                                    partition_id_tensor)

    install_neuronx_cc_hook()

    partition_name = (nc.partition_id_tensor.name
                      if nc.partition_id_tensor else None)
    in_names, out_names, out_avals = [], [], []
    for alloc in nc.m.functions[0].allocations:
        if not isinstance(alloc, mybir.MemoryLocationSet):
            continue
        if not alloc.memorylocations:
            continue
        name = alloc.memorylocations[0].name
        if alloc.kind == "ExternalInput":
            if name != partition_name:
                in_names.append(name)
        elif alloc.kind == "ExternalOutput":
            out_names.append(name)
            out_avals.append(jax.core.ShapedArray(
                tuple(alloc.tensor_shape), mybir.dt.np(alloc.dtype)))
    n_params = len(in_names)
    n_outs = len(out_names)
    zero_shapes = [(a.shape, a.dtype) for a in out_avals]
    all_in_names = in_names + out_names
    if partition_name is not None:
        all_in_names = all_in_names + [partition_name]

    def _body(*args):
        operands = list(args)
        if partition_name is not None:
            operands.append(partition_id_tensor())
        outs = _bass_exec_p.bind(
            *operands,
            out_avals=tuple(out_avals),
            in_names=tuple(all_in_names),
            out_names=tuple(out_names),
            lowering_input_output_aliases=(),
            sim_require_finite=True,
            sim_require_nnan=True,
            nc=nc,
        )
        return tuple(outs)

    devices = jax.devices()[:NCORES]
    mesh = Mesh(np.asarray(devices), ("core",))
    sharded = jax.jit(
        shard_map(_body, mesh=mesh,
                  in_specs=(PartitionSpec("core"),) * (n_params + n_outs),
                  out_specs=(PartitionSpec("core"),) * n_outs,
                  check_rep=False),
        keep_unused=True,
    )

    def run(in_maps):
        concat_in = [
            _np.concatenate([_np.asarray(in_maps[c][n]) for c in range(NCORES)],
                            axis=0)
            for n in in_names
        ]
        concat_zeros = [
            _np.zeros((NCORES * s[0], *s[1:]), d) for (s, d) in zero_shapes
        ]
        out_arrs = sharded(*concat_in, *concat_zeros)
        return [
            {
                n: _np.asarray(out_arrs[i]).reshape(NCORES, *out_avals[i].shape)[c]
                for i, n in enumerate(out_names)
            }
            for c in range(NCORES)
        ]

    internals = dict(nc=nc, body=_body, mesh=mesh, in_names=in_names,
                     out_names=out_names, zero_shapes=zero_shapes,
                     n_params=n_params)
    return run, in_names, internals


def _get_runner():
    """Build the Bass program once and return a cached 8-core PJRT callable."""
    global _RUNNER, _INTERNALS
    if _RUNNER is not None:
        return _RUNNER
    run, in_names, internals = _make_pjrt_runner(_build_program())
    _INTERNALS = internals
    _RUNNER = (run, in_names)
    return _RUNNER


def _make_in_maps(x, wq, wk, wv, wo):
    x = np.asarray(x, np.float32)
    wq_s = np.asarray(wq, np.float32) * (1.0 / np.sqrt(HD))  # fold score scale
    wk = np.asarray(wk, np.float32)
    wv = np.asarray(wv, np.float32)
    wo = np.asarray(wo, np.float32)

    xt_b = [_round_f32r(x[b].T) for b in range(B)]
    in_maps = []
    for c in range(NCORES):
        b, g = c // GROUPS, c % GROUPS
        sl = slice(DH * g, DH * (g + 1))
        in_maps.append({
            "xt": xt_b[b],
            "wqt": _round_f32r(wq_s[sl, :].T),
            "wkt": _round_f32r(wk[sl, :].T),
            "wvt": _round_f32r(wv[sl, :].T),
            "wot": _round_f32r(wo[:, sl].T),
        })
    return in_maps


def kernel(x, wq, wk, wv, wo):
    run, _ = _get_runner()
    results = run(_make_in_maps(x, wq, wk, wv, wo))
    y = np.zeros((B, T, DIM), np.float32)
    for c in range(NCORES):
        y[c // GROUPS] += results[c]["y"]
    return y
